# revision 13
# baseline (speedup 1.0000x reference)
import numpy as np
import concourse.bacc as bacc
import concourse.bass as bass
import concourse.mybir as mybir
from concourse.bass_utils import run_bass_kernel_spmd

DIM_INPUT = 128
DIM_REC = 512
DIM_OUT = 256
BATCH = 512
NCORES = 8
B = BATCH // NCORES  # 64 per-core batch
T = DIM_INPUT        # 128 timesteps
KJ = DIM_REC // 128  # 4 chunks of the recurrent dim
OJ = DIM_OUT // 128  # 2 chunks of the output dim

F32 = mybir.dt.float32
MMDT = mybir.dt.float16
MMNP = np.float16

# Packed-wxx column layout (fp16 columns): Wx.T | x.T | bc bits | by bits
WXC = DIM_REC            # 512: end of WxT
XTC = WXC + B            # 576: end of xT
BCC = XTC + 2 * KJ       # 584: end of bc (4 fp32 = 8 fp16 cols)
BYC = BCC + 2 * OJ       # 588: end of by (2 fp32 = 4 fp16 cols)

# Steady-state MM issue order per step, from discrete-event search
# (sched_search.py; model period 921ns, measured 857ns/step).
# ('s',q) = x-projection seed for psum group q (start=True);
# (q,k) accumulates Wh[k->q] @ g_k.
ORDER = [('s', 2), ('s', 0), ('s', 3), ('s', 1),
         (2, 0), (0, 2), (2, 2), (3, 0), (0, 0), (1, 2), (0, 1), (0, 3),
         (2, 3), (2, 1), (1, 0), (3, 1), (1, 3), (1, 1), (3, 2), (3, 3)]
EPI_S = [0, 1]   # scalar-engine epilogue groups, in FIFO order
EPI_V = [2, 3]   # vector-engine epilogue groups, in FIFO order
NWARM = 6        # junk wide matmuls to lift the PE HAM clock-gate early

# position (0-based) of the last writer of each psum group within ORDER
LAST_W = {q: max(i for i, t in enumerate(ORDER)
                 if (t[0] == 's' and t[1] == q) or (t[0] != 's' and t[0] == q))
          for q in range(4)}
# position of the first consumer of g_k within ORDER
FIRST_C = {k: min(i for i, t in enumerate(ORDER) if t[0] != 's' and t[1] == k)
           for k in range(4)}


def _build_nc():
    nc = bacc.Bacc("TRN2", target_bir_lowering=False, debug=False,
                   num_devices=NCORES)
    wxx = nc.dram_tensor("wxx", [128, BYC], MMDT, kind="ExternalInput")
    WhT = nc.dram_tensor("WhT", [DIM_REC, DIM_REC], MMDT, kind="ExternalInput")
    whyR = nc.dram_tensor("whyR", [128, KJ * DIM_OUT], MMDT, kind="ExternalInput")
    yT = nc.dram_tensor("yT", [DIM_OUT, B], F32, kind="ExternalOutput")

    RELU = mybir.ActivationFunctionType.Relu
    IDENT = mybir.ActivationFunctionType.Identity
    ADD = mybir.AluOpType.add
    MAX = mybir.AluOpType.max

    from contextlib import ExitStack
    with ExitStack() as ctx:
        def sb(name, shape, dt):
            return ctx.enter_context(nc.sbuf_tensor(name, shape, dt))

        def psb(name):
            return ctx.enter_context(nc.psum_tensor(name, [128, 512], F32))

        def sem(name):
            return ctx.enter_context(nc.semaphore(name))

        wh = [sb(f"wh{k}", [128, DIM_REC], MMDT) for k in range(KJ)]
        wxt = sb("wxs", [128, BYC], MMDT)
        whyt = sb("why", [128, KJ * DIM_OUT], MMDT)
        g = [[sb(f"g{p}{k}", [128, B], MMDT) for k in range(KJ)]
             for p in range(2)]
        yt0 = sb("yt0", [128, B], F32)
        yt1 = sb("yt1", [128, B], F32)
        ps = [[psb(f"p{p}{q}") for q in range(KJ)] for p in range(2)]
        mm = sem("mm")
        gsem = [sem(f"gs{q}") for q in range(KJ)]
        gs0, gs1 = gsem[0], gsem[1]
        dsy = sem("dsy")      # sync-queue dma completions
        dsc = sem("dsc")      # scalar-queue dma completions
        dgp = sem("dgp")      # gpsimd-queue dma completions

        def wxs(q):           # Wx.T column block q (stationary operand)
            return wxt[:, q * 128:(q + 1) * 128]

        xta = wxt[:, WXC:XTC]

        def bca(q):           # bc[q] as a [128,1] fp32 per-partition bias
            return wxt[:, XTC + 2 * q:XTC + 2 * q + 2].bitcast(F32)

        def bya(j):
            return wxt[:, BCC + 2 * j:BCC + 2 * j + 2].bitcast(F32)

        # mm-semaphore count after step s (s>=1):  4 + 20*s
        def base(s):
            return 4 + 20 * (s - 1)

        with nc.Block() as block:

            @block.sync
            def _(sync):
                sync.dma_start(out=wxt[0:64, :], in_=wxx[0:64, :]).then_inc(dsy, 16)
                sync.dma_start(out=wh[2][:], in_=WhT[256:384, :]).then_inc(dsy, 16)
                # output: first half of y (gs0 hits T+1 only via the final
                # scalar ACT that produces yt0)
                sync.wait_ge(gs0, T + 1)
                sync.dma_start(out=yT[0:128, :], in_=yt0[:]).then_inc(dsy, 16)
                sync.wait_ge(dsy, 48)

            @block.gpsimd
            def _(gpsimd):
                gpsimd.dma_start(out=wh[0][:], in_=WhT[0:128, :]).then_inc(dgp, 16)
                gpsimd.dma_start(out=wh[1][:], in_=WhT[128:256, :]).then_inc(dgp, 16)
                gpsimd.dma_start(out=whyt[:], in_=whyR[:]).then_inc(dgp, 16)
                # output: second half of y (gs1 hits T+1 only via the final
                # vector tensor_scalar that produces yt1)
                gpsimd.wait_ge(gs1, T + 1)
                gpsimd.dma_start(out=yT[128:256, :], in_=yt1[:]).then_inc(dgp, 16)
                gpsimd.wait_ge(dgp, 64)

            @block.tensor
            def _(tensor):
                # junk matmuls on uninitialized sbuf: keep the PE busy during
                # the weight DMA so the HAM clock-gate lifts before step 0
                for _ in range(NWARM):
                    tensor.matmul(ps[1][0][0:B, :], g[1][0][:],
                                  whyt[:, 0:512], start=True, stop=True)

                # step 0: h0 == 0, so psum = x @ Wx only; seed order from ORDER
                seed_q = [t[1] for t in ORDER if t[0] == 's']
                tensor.wait_ge(dsy, 16)
                tensor.wait_ge(dsc, 16)
                for q in seed_q:
                    tensor.matmul(ps[0][q][:, 0:B], wxs(q),
                                  xta, start=True, stop=True).then_inc(mm)

                # steps 1..T-1
                for s in range(1, T):
                    cur = g[(s + 1) % 2]   # g written by step s-1
                    pc = ps[s % 2]
                    grp = [0] * KJ
                    for i, t in enumerate(ORDER):
                        if t[0] == 's':
                            q = t[1]
                            tensor.matmul(pc[q][:, 0:B], wxs(q),
                                          xta, start=True,
                                          stop=False).then_inc(mm)
                        else:
                            q, k = t
                            if i == FIRST_C[k]:
                                tensor.wait_ge(gsem[k], s)
                                if s == 1:
                                    # first use of wh[k]: its dma must be done
                                    dmaw = [(dgp, 16), (dgp, 32),
                                            (dsy, 32), (dsc, 32)][k]
                                    tensor.wait_ge(*dmaw)
                            grp[q] += 1
                            tensor.matmul(pc[q][:, 0:B],
                                          wh[k][:, q * 128:(q + 1) * 128],
                                          cur[k][:], start=False,
                                          stop=(grp[q] == KJ)).then_inc(mm)

                # output layer: yT[j] = Why[j] @ h + by[j]
                gfin = g[(T - 1) % 2]
                tensor.wait_ge(dgp, 48)
                for j in range(OJ):
                    for k in range(KJ):
                        if j == 0:
                            tensor.wait_ge(gsem[k], T)
                        tensor.matmul(
                            ps[0][j][:, 0:B],
                            whyt[:, k * DIM_OUT + j * 128:k * DIM_OUT + (j + 1) * 128],
                            gfin[k][:], start=(k == 0),
                            stop=(k == KJ - 1)).then_inc(mm)

            @block.scalar
            def _(scalar):
                scalar.dma_start(out=wxt[64:128, :],
                                 in_=wxx[64:128, :]).then_inc(dsc, 16)
                scalar.dma_start(out=wh[3][:], in_=WhT[384:512, :]).then_inc(dsc, 16)
                # step 0 epilogues (groups EPI_S); wxx load is implied by the
                # seed matmuls having completed (mm counts)
                seed_q = [t[1] for t in ORDER if t[0] == 's']
                for q in EPI_S:
                    scalar.wait_ge(mm, seed_q.index(q) + 1)
                    scalar.activation(g[0][q][:], ps[0][q][:, 0:B], RELU,
                                      bias=bca(q)).then_inc(gsem[q])
                for s in range(1, T):
                    nxt = g[s % 2]
                    pc = ps[s % 2]
                    for q in EPI_S:
                        scalar.wait_ge(mm, base(s) + LAST_W[q] + 1)
                        scalar.activation(nxt[q][:], pc[q][:, 0:B], RELU,
                                          bias=bca(q)).then_inc(gsem[q])
                # final output epilogue, first half
                scalar.wait_ge(mm, 4 + 20 * (T - 1) + 4)
                scalar.activation(yt0[:], ps[0][0][:, 0:B], IDENT,
                                  bias=bya(0)).then_inc(gs0)

            @block.vector
            def _(vector):
                seed_q = [t[1] for t in ORDER if t[0] == 's']
                for q in EPI_V:
                    vector.wait_ge(mm, seed_q.index(q) + 1)
                    vector.tensor_scalar(g[0][q][:], ps[0][q][:, 0:B],
                                         bca(q), 0.0, ADD,
                                         MAX).then_inc(gsem[q])
                for s in range(1, T):
                    nxt = g[s % 2]
                    pc = ps[s % 2]
                    for q in EPI_V:
                        vector.wait_ge(mm, base(s) + LAST_W[q] + 1)
                        vector.tensor_scalar(nxt[q][:], pc[q][:, 0:B],
                                             bca(q), 0.0, ADD,
                                             MAX).then_inc(gsem[q])
                # final output epilogue, second half
                vector.wait_ge(mm, 4 + 20 * (T - 1) + 8)
                vector.tensor_scalar(yt1[:], ps[0][1][:, 0:B], bya(1),
                                     None, ADD).then_inc(gs1)

    nc.compile()
    return nc


_NC = None
TRACE = False
TRACE_TMPDIR = None
LAST_RESULTS = None


def kernel(x, W_x2h, b_x2h, W_h2h, b_h2h, W_h2y, b_h2y):
    global _NC, LAST_RESULTS
    if _NC is None:
        _NC = _build_nc()

    x = np.asarray(x, np.float32)
    WhyT = np.asarray(W_h2y, np.float32).T.astype(MMNP)
    bc = np.asarray(b_x2h, np.float32) + np.asarray(b_h2h, np.float32)
    bcR = np.ascontiguousarray(bc.reshape(KJ, 128).T)              # [128,4] f32
    byR = np.ascontiguousarray(
        np.asarray(b_h2y, np.float32).reshape(OJ, 128).T)          # [128,2] f32
    WxTn = np.asarray(W_x2h, np.float32).T.astype(MMNP)            # [128,512]
    shared = {
        "WhT": np.ascontiguousarray(np.asarray(W_h2h, np.float32).T.astype(MMNP)),
        "whyR": np.ascontiguousarray(np.concatenate(
            [WhyT[k * 128:(k + 1) * 128, :] for k in range(KJ)], axis=1)),
    }
    ins = []
    for i in range(NCORES):
        m = dict(shared)
        wxxn = np.empty((128, BYC), MMNP)
        wxxn[:, 0:WXC] = WxTn
        wxxn[:, WXC:XTC] = x[i * B:(i + 1) * B, :].T.astype(MMNP)
        wxxn[:, XTC:BCC] = bcR.view(MMNP)
        wxxn[:, BCC:BYC] = byR.view(MMNP)
        m["wxx"] = np.ascontiguousarray(wxxn)
        ins.append(m)

    kw = {}
    if TRACE:
        kw = {"trace": True, "tmpdir": TRACE_TMPDIR}
    res = run_bass_kernel_spmd(_NC, ins, core_ids=list(range(NCORES)), **kw)
    LAST_RESULTS = res
    out = np.empty((BATCH, DIM_OUT), np.float32)
    for i in range(NCORES):
        out[i * B:(i + 1) * B, :] = res.results[i]["yT"].T
    return out


# revision 14
# speedup vs baseline: 1.0144x; 1.0144x over previous
import numpy as np
import concourse.bacc as bacc
import concourse.bass as bass
import concourse.mybir as mybir
from concourse.bass_utils import run_bass_kernel_spmd

DIM_INPUT = 128
DIM_REC = 512
DIM_OUT = 256
BATCH = 512
NCORES = 8
B = BATCH // NCORES  # 64 per-core batch
T = DIM_INPUT        # 128 timesteps
KJ = DIM_REC // 128  # 4 chunks of the recurrent dim
OJ = DIM_OUT // 128  # 2 chunks of the output dim

F32 = mybir.dt.float32
MMDT = mybir.dt.float16
MMNP = np.float16

# Packed-wxx column layout (fp16 columns): Wx.T | x.T | bc bits | by bits
WXC = DIM_REC            # 512: end of WxT
XTC = WXC + B            # 576: end of xT
BCC = XTC + 2 * KJ       # 584: end of bc (4 fp32 = 8 fp16 cols)
BYC = BCC + 2 * OJ       # 588: end of by (2 fp32 = 4 fp16 cols)

# Steady-state MM issue order per step, from discrete-event search
# (sched_search.py; model period 921ns, measured 857ns/step).
# ('s',q) = x-projection seed for psum group q (start=True);
# (q,k) accumulates Wh[k->q] @ g_k.
ORDER = [('s', 2), ('s', 0), ('s', 3), ('s', 1),
         (2, 0), (0, 2), (2, 2), (3, 0), (0, 0), (1, 2), (0, 1), (0, 3),
         (2, 3), (2, 1), (1, 0), (3, 1), (1, 3), (1, 1), (3, 2), (3, 3)]
EPI_S = [0, 1]   # scalar-engine epilogue groups, in FIFO order
EPI_V = [2, 3]   # vector-engine epilogue groups, in FIFO order
NWARM = 6        # junk wide matmuls to lift the PE HAM clock-gate early

# position (0-based) of the last writer of each psum group within ORDER
LAST_W = {q: max(i for i, t in enumerate(ORDER)
                 if (t[0] == 's' and t[1] == q) or (t[0] != 's' and t[0] == q))
          for q in range(4)}
# position of the first consumer of g_k within ORDER
FIRST_C = {k: min(i for i, t in enumerate(ORDER) if t[0] != 's' and t[1] == k)
           for k in range(4)}


def _build_nc():
    nc = bacc.Bacc("TRN2", target_bir_lowering=False, debug=False,
                   num_devices=NCORES)
    wxx = nc.dram_tensor("wxx", [128, BYC], MMDT, kind="ExternalInput")
    WhT = nc.dram_tensor("WhT", [DIM_REC, DIM_REC], MMDT, kind="ExternalInput")
    whyR = nc.dram_tensor("whyR", [128, KJ * DIM_OUT], MMDT, kind="ExternalInput")
    yT = nc.dram_tensor("yT", [DIM_OUT, B], F32, kind="ExternalOutput")

    RELU = mybir.ActivationFunctionType.Relu
    IDENT = mybir.ActivationFunctionType.Identity
    ADD = mybir.AluOpType.add
    MAX = mybir.AluOpType.max

    from contextlib import ExitStack
    with ExitStack() as ctx:
        def sb(name, shape, dt):
            return ctx.enter_context(nc.sbuf_tensor(name, shape, dt))

        def psb(name):
            return ctx.enter_context(nc.psum_tensor(name, [128, 512], F32))

        def sem(name):
            return ctx.enter_context(nc.semaphore(name))

        wh = [sb(f"wh{k}", [128, DIM_REC], MMDT) for k in range(KJ)]
        wxt = sb("wxs", [128, BYC], MMDT)
        whyt = sb("why", [128, KJ * DIM_OUT], MMDT)
        g = [[sb(f"g{p}{k}", [128, B], MMDT) for k in range(KJ)]
             for p in range(2)]
        yt0 = sb("yt0", [128, B], F32)
        yt1 = sb("yt1", [128, B], F32)
        ps = [[psb(f"p{p}{q}") for q in range(KJ)] for p in range(2)]
        mm = sem("mm")
        gsem = [sem(f"gs{q}") for q in range(KJ)]
        gs0, gs1 = gsem[0], gsem[1]
        dsy = sem("dsy")      # sync-queue dma completions
        dsc = sem("dsc")      # scalar-queue dma completions
        dgp = sem("dgp")      # gpsimd-queue dma completions

        def wxs(q):           # Wx.T column block q (stationary operand)
            return wxt[:, q * 128:(q + 1) * 128]

        xta = wxt[:, WXC:XTC]

        def bca(q):           # bc[q] as a [128,1] fp32 per-partition bias
            return wxt[:, XTC + 2 * q:XTC + 2 * q + 2].bitcast(F32)

        def bya(j):
            return wxt[:, BCC + 2 * j:BCC + 2 * j + 2].bitcast(F32)

        # mm-semaphore count after step s (s>=1):  4 + 20*s
        def base(s):
            return 4 + 20 * (s - 1)

        with nc.Block() as block:

            @block.sync
            def _(sync):
                sync.dma_start(out=wxt[0:64, :], in_=wxx[0:64, :]).then_inc(dsy, 16)
                sync.dma_start(out=wh[2][:], in_=WhT[256:384, :]).then_inc(dsy, 16)
                # output: first half of y (gs0 hits T+1 only via the final
                # scalar ACT that produces yt0)
                sync.wait_ge(gs0, T + 1)
                sync.dma_start(out=yT[0:128, :], in_=yt0[:]).then_inc(dsy, 16)
                sync.wait_ge(dsy, 48)

            @block.gpsimd
            def _(gpsimd):
                gpsimd.dma_start(out=wh[0][:], in_=WhT[0:128, :]).then_inc(dgp, 16)
                gpsimd.dma_start(out=wh[1][:], in_=WhT[128:256, :]).then_inc(dgp, 16)
                gpsimd.dma_start(out=whyt[:], in_=whyR[:]).then_inc(dgp, 16)
                # output: second half of y (gs1 hits T+1 only via the final
                # vector tensor_scalar that produces yt1)
                gpsimd.wait_ge(gs1, T + 1)
                gpsimd.dma_start(out=yT[128:256, :], in_=yt1[:]).then_inc(dgp, 16)
                gpsimd.wait_ge(dgp, 64)

            @block.tensor
            def _(tensor):
                # junk matmuls on uninitialized sbuf: keep the PE busy during
                # the weight DMA so the HAM clock-gate lifts before step 0
                for _ in range(NWARM):
                    tensor.matmul(ps[1][0][0:B, :], g[1][0][:],
                                  whyt[:, 0:512], start=True, stop=True)

                # step 0: h0 == 0, so psum = x @ Wx only; seed order from ORDER
                seed_q = [t[1] for t in ORDER if t[0] == 's']
                tensor.wait_ge(dsy, 16)
                tensor.wait_ge(dsc, 16)
                for q in seed_q:
                    tensor.matmul(ps[0][q][:, 0:B], wxs(q),
                                  xta, start=True, stop=True).then_inc(mm)

                # more junk matmuls: bridge the idle window while the wh
                # DMAs land, so the HAM gate does not re-throttle.  They
                # write ps[1][3], which step 1's seed clears (start=True).
                for _ in range(NWARM):
                    tensor.matmul(ps[1][3][0:B, :], g[1][0][:],
                                  whyt[:, 0:512], start=True, stop=True)

                # steps 1..T-1
                for s in range(1, T):
                    cur = g[(s + 1) % 2]   # g written by step s-1
                    pc = ps[s % 2]
                    grp = [0] * KJ
                    for i, t in enumerate(ORDER):
                        if t[0] == 's':
                            q = t[1]
                            tensor.matmul(pc[q][:, 0:B], wxs(q),
                                          xta, start=True,
                                          stop=False).then_inc(mm)
                        else:
                            q, k = t
                            if i == FIRST_C[k]:
                                tensor.wait_ge(gsem[k], s)
                                if s == 1:
                                    # first use of wh[k]: its dma must be done
                                    dmaw = [(dgp, 16), (dgp, 32),
                                            (dsy, 32), (dsc, 32)][k]
                                    tensor.wait_ge(*dmaw)
                            grp[q] += 1
                            tensor.matmul(pc[q][:, 0:B],
                                          wh[k][:, q * 128:(q + 1) * 128],
                                          cur[k][:], start=False,
                                          stop=(grp[q] == KJ)).then_inc(mm)

                # output layer: yT[j] = Why[j] @ h + by[j]
                gfin = g[(T - 1) % 2]
                tensor.wait_ge(dgp, 48)
                for j in range(OJ):
                    for k in range(KJ):
                        if j == 0:
                            tensor.wait_ge(gsem[k], T)
                        tensor.matmul(
                            ps[0][j][:, 0:B],
                            whyt[:, k * DIM_OUT + j * 128:k * DIM_OUT + (j + 1) * 128],
                            gfin[k][:], start=(k == 0),
                            stop=(k == KJ - 1)).then_inc(mm)

            @block.scalar
            def _(scalar):
                scalar.dma_start(out=wxt[64:128, :],
                                 in_=wxx[64:128, :]).then_inc(dsc, 16)
                scalar.dma_start(out=wh[3][:], in_=WhT[384:512, :]).then_inc(dsc, 16)
                # step 0 epilogues (groups EPI_S); wxx load is implied by the
                # seed matmuls having completed (mm counts)
                seed_q = [t[1] for t in ORDER if t[0] == 's']
                for q in EPI_S:
                    scalar.wait_ge(mm, seed_q.index(q) + 1)
                    scalar.activation(g[0][q][:], ps[0][q][:, 0:B], RELU,
                                      bias=bca(q)).then_inc(gsem[q])
                for s in range(1, T):
                    nxt = g[s % 2]
                    pc = ps[s % 2]
                    for q in EPI_S:
                        scalar.wait_ge(mm, base(s) + LAST_W[q] + 1)
                        scalar.activation(nxt[q][:], pc[q][:, 0:B], RELU,
                                          bias=bca(q)).then_inc(gsem[q])
                # final output epilogue, first half
                scalar.wait_ge(mm, 4 + 20 * (T - 1) + 4)
                scalar.activation(yt0[:], ps[0][0][:, 0:B], IDENT,
                                  bias=bya(0)).then_inc(gs0)

            @block.vector
            def _(vector):
                seed_q = [t[1] for t in ORDER if t[0] == 's']
                for q in EPI_V:
                    vector.wait_ge(mm, seed_q.index(q) + 1)
                    vector.tensor_scalar(g[0][q][:], ps[0][q][:, 0:B],
                                         bca(q), 0.0, ADD,
                                         MAX).then_inc(gsem[q])
                for s in range(1, T):
                    nxt = g[s % 2]
                    pc = ps[s % 2]
                    for q in EPI_V:
                        vector.wait_ge(mm, base(s) + LAST_W[q] + 1)
                        vector.tensor_scalar(nxt[q][:], pc[q][:, 0:B],
                                             bca(q), 0.0, ADD,
                                             MAX).then_inc(gsem[q])
                # final output epilogue, second half
                vector.wait_ge(mm, 4 + 20 * (T - 1) + 8)
                vector.tensor_scalar(yt1[:], ps[0][1][:, 0:B], bya(1),
                                     None, ADD).then_inc(gs1)

    nc.compile()
    return nc


_NC = None
TRACE = False
TRACE_TMPDIR = None
LAST_RESULTS = None


def kernel(x, W_x2h, b_x2h, W_h2h, b_h2h, W_h2y, b_h2y):
    global _NC, LAST_RESULTS
    if _NC is None:
        _NC = _build_nc()

    x = np.asarray(x, np.float32)
    WhyT = np.asarray(W_h2y, np.float32).T.astype(MMNP)
    bc = np.asarray(b_x2h, np.float32) + np.asarray(b_h2h, np.float32)
    bcR = np.ascontiguousarray(bc.reshape(KJ, 128).T)              # [128,4] f32
    byR = np.ascontiguousarray(
        np.asarray(b_h2y, np.float32).reshape(OJ, 128).T)          # [128,2] f32
    WxTn = np.asarray(W_x2h, np.float32).T.astype(MMNP)            # [128,512]
    shared = {
        "WhT": np.ascontiguousarray(np.asarray(W_h2h, np.float32).T.astype(MMNP)),
        "whyR": np.ascontiguousarray(np.concatenate(
            [WhyT[k * 128:(k + 1) * 128, :] for k in range(KJ)], axis=1)),
    }
    ins = []
    for i in range(NCORES):
        m = dict(shared)
        wxxn = np.empty((128, BYC), MMNP)
        wxxn[:, 0:WXC] = WxTn
        wxxn[:, WXC:XTC] = x[i * B:(i + 1) * B, :].T.astype(MMNP)
        wxxn[:, XTC:BCC] = bcR.view(MMNP)
        wxxn[:, BCC:BYC] = byR.view(MMNP)
        m["wxx"] = np.ascontiguousarray(wxxn)
        ins.append(m)

    kw = {}
    if TRACE:
        kw = {"trace": True, "tmpdir": TRACE_TMPDIR}
    res = run_bass_kernel_spmd(_NC, ins, core_ids=list(range(NCORES)), **kw)
    LAST_RESULTS = res
    out = np.empty((BATCH, DIM_OUT), np.float32)
    for i in range(NCORES):
        out[i * B:(i + 1) * B, :] = res.results[i]["yT"].T
    return out
